# revision 16
# baseline (speedup 1.0000x reference)
"""Trainium2 Bass kernel for nn_GAT_7086696039040 (2-layer GAT over 64 dense
128-node graphs + global BatchNorms + FCN head), data-parallel over 8 cores.

Structure exploited (verified against the reference's setup_inputs):
  - edge_index is the dense per-graph block pattern (every graph fully
    connected), so segment-softmax attention == dense 128x128 softmax per
    graph; edge_attr / edge_index / batch never touch the device.
  - attention logits are rank-1 structured: e = lrelu(a_s[src] + a_d[dst]);
    a_s / a_d fold into the layer weight matrix as 4 extra output columns.
  - softmax denominator division defers past the aggregation matmul.
  - GATConv bias folds into the following linear layer's bias.
  - the two BatchNorms need global (all-core) mean/var -> two tiny AllReduces.

Self-contained: hardcodes all shapes; falls back to a numpy reference if the
edge structure is not the expected dense block pattern.
"""

import sys

sys.path.insert(0, "/opt/trn_rl_repo")

import os

import numpy as np

import concourse.bass as bass
import concourse.bacc as bacc
import concourse.tile as tile
import concourse.mybir as mybir
import concourse.bass_utils as bass_utils

AF = mybir.ActivationFunctionType
ALU = mybir.AluOpType
F16 = mybir.dt.float16
F32 = mybir.dt.float32

N_CORES = 8
P = 128            # nodes per graph
G = 8              # graphs per core
B = 64             # total graphs
NPC = P * G        # nodes per core (1024)
NTOT = P * B       # total nodes (8192)
H, C = 2, 64
EPS, NEG = 1e-5, 0.2

# const DRAM tensors: name -> (shape, dtype)
CONST_SPECS = {
    "w0aug": ([128, 134], F16),
    "w1aug": ([64, 134], F16),
    "wpt": ([128, 64], F16),
    "wmt": ([128, 64], F16),
    "wnt": ([64, 8], F16),
    "wf1r": ([128, 2048], F16),
    "wf2t": ([128, 64], F16),
    "wf3t": ([32, 1], F16),
    "bf1row": ([1, 256], F16),
    "ones_row": ([1, 128], F16),
    "ones_col": ([128, 1], F16),
    "ident": ([128, 128], F16),
    "bpeff": ([64, 1], F32),
    "bpeff02": ([64, 1], F32),
    "bmeff": ([64, 1], F32),
    "bmeff02": ([64, 1], F32),
    "bnode": ([8, 1], F32),
    "bnode02": ([8, 1], F32),
    "bf2": ([32, 1], F32),
    "bf202": ([32, 1], F32),
    "bf3": ([1, 1], F32),
    "bn0g": ([64, 1], F32),
    "bn0b": ([64, 1], F32),
    "bn1g": ([8, 1], F32),
    "bn1b": ([8, 1], F32),
    "epsc": ([64, 1], F32),
}


def build_program(use_cc=True, reps=1):
    """Build the per-core Bass program. reps>1 unrolls the whole body for
    in-NEFF timing (amortizes dispatch overhead)."""
    nc = bacc.Bacc("TRN2", target_bir_lowering=False, debug=False,
                   num_devices=N_CORES)

    dins = {"xt16": nc.dram_tensor("xt16", [128, NPC], F16, kind="ExternalInput")}
    for name, (shape, dt) in CONST_SPECS.items():
        dins[name] = nc.dram_tensor(name, shape, dt, kind="ExternalInput")
    y_d = nc.dram_tensor("y", [1, G], F32, kind="ExternalOutput")
    dbg_d = nc.dram_tensor("dbg", [128, 1024], F32, kind="ExternalOutput")
    nc._dbg_d = dbg_d

    with tile.TileContext(nc) as tc:
        with tc.tile_pool(name="const", bufs=1) as cp, \
             tc.tile_pool(name="work", bufs=3) as wk, \
             tc.tile_pool(name="acc", bufs=1) as ac, \
             tc.tile_pool(name="psum", bufs=1, space="PSUM") as ps, \
             tc.tile_pool(name="dram", bufs=1, space="DRAM") as dr:

            sb = {}
            for name, t in dins.items():
                tl = cp.tile(t.shape, t.dtype, tag=name)
                nc.sync.dma_start(tl[:], t.ap())
                sb[name] = tl

            for rep in range(reps):
                _emit_body(nc, tc, sb, wk, ac, ps, dr, y_d, use_cc, rep)

    nc.compile()
    return nc


def _emit_body(nc, tc, sb, wk, ac, ps, dr, y_d, use_cc, rep):
    STAGE = int(os.environ.get("K_STAGE", "5"))
    DBG = os.environ.get("K_DBG", "")
    dbg_done = [False]

    def dbg_dump(name, ap, rows, cols):
        if DBG != name or dbg_done[0]:
            return
        dbg_done[0] = True
        t = wk.tile([rows, cols], F32, tag="dbgt")
        nc.vector.tensor_copy(t[:], ap)
        nc.sync.dma_start(nc._dbg_d.ap()[0:rows, 0:cols], t[:])
    t0_all = u0_all = t1_all = a1_all = None
    if 1 <= STAGE <= 5:
        t0_all = ac.tile([64, NPC], F16, tag="t0all")
    if 4 <= STAGE <= 5:
        u0_all = ac.tile([64, NPC], F16, tag="u0all")
    if STAGE == 5:
        t1_all = ac.tile([8, NPC], F16, tag="t1all")
        a1_all = ac.tile([8, NPC], F16, tag="a1all")

    def gat_layer(li, lhs_src):
        p0h = 64
        waug = sb["w0aug" if li == 0 else "w1aug"]
        wlin = sb["wpt" if li == 0 else "wmt"]
        blin = sb["bpeff" if li == 0 else "bmeff"]
        for g in range(G):
            hps = ps.tile([128, 134], F32, tag="hps")
            nc.tensor.matmul(hps[:], lhs_src(g), waug[:], start=True, stop=True)
            if STAGE == 90:   # matmul only
                if g > 0:
                    continue
                yo = wk.tile([1, G], F32, tag="yo90")
                nc.vector.tensor_copy(yo[:], hps[0:1, 0:G])
                nc.sync.dma_start(y_d.ap(), yo[:])
                continue
            hsb = wk.tile([128, 128], F16, tag="hsb")
            nc.vector.tensor_copy(hsb[:], hps[:, 0:128])
            if STAGE == 91:   # matmul + hsb copy
                if g > 0:
                    continue
                yo = wk.tile([1, G], F32, tag="yo91")
                nc.vector.tensor_copy(yo[:], hsb[0:1, 0:G])
                nc.sync.dma_start(y_d.ap(), yo[:])
                continue
            if STAGE <= 1:
                nc.vector.tensor_copy(t0_all[:p0h, P * g:P * (g + 1)],
                                      hps[0:p0h, 0:128])
                continue
            ssb = wk.tile([128, 6], F32, tag="ssb")
            nc.vector.tensor_copy(ssb[:], hps[:, 128:134])
            dsb = wk.tile([128, 2], F16, tag="dsb")
            nc.vector.tensor_copy(dsb[:], hps[:, 130:132])
            dbg_dump(f"hsb{li}", hps[:, 0:128], 128, 128)
            dbg_dump(f"ssb{li}", hps[:, 128:134], 128, 6)
            sdt = ps.tile([1, 256], F16, tag="sdt")
            drows = []
            for h in range(2):
                nc.tensor.transpose(sdt[:, 128 * h:128 * (h + 1)],
                                    dsb[:, h:h + 1], sb["ident"][:])
                dr_h = wk.tile([1, 128], F16, tag=f"drow{h}")
                nc.vector.tensor_copy(dr_h[:], sdt[:, 128 * h:128 * (h + 1)])
                drows.append(dr_h)
            zps = ps.tile([128, 128], F32, tag="zps")
            dn = ps.tile([1, 256], F32, tag="dn")
            rb0 = wk.tile([64, 128], F32, tag="rb0")
            rb1 = wk.tile([64, 128], F32, tag="rb1")
            rbs = [rb0, rb1]
            for h in range(2):
                fps = ps.tile([128, 128], F32, tag="fps")
                # F[j, i] = d_i (rank-1 ones x d_row broadcast)
                nc.tensor.matmul(fps[:], sb["ones_row"][:], drows[h][:],
                                 start=True, stop=True)
                # exp(lrelu(z)) == max(exp(z), exp(0.2 z)); z = F + s_j
                ex1 = wk.tile([128, 128], F16, tag="ex1")
                nc.scalar.activation(ex1[:], fps[:], AF.Exp,
                                     bias=ssb[:, h:h + 1])
                ex2 = wk.tile([128, 128], F16, tag="ex2")
                nc.scalar.activation(ex2[:], fps[:], AF.Exp, scale=NEG,
                                     bias=ssb[:, 4 + h:5 + h])
                ex = wk.tile([128, 128], F16, tag="ex")
                nc.vector.tensor_tensor(ex[:], ex1[:], ex2[:], op=ALU.max)
                dbg_dump(f"ex{li}", ex[:], 128, 128)
                dbg_dump(f"fps{li}", fps[:], 128, 128)
                if STAGE <= 2:
                    if h == 0:
                        nc.vector.tensor_copy(t0_all[:p0h, P * g:P * (g + 1)],
                                              ex[0:p0h, :])
                    continue
                # feature-major aggregation: out[c, i] = sum_j h[j, c] EX[j, i]
                nc.tensor.matmul(zps[64 * h:64 * (h + 1), :],
                                 hsb[:, 64 * h:64 * (h + 1)], ex[:],
                                 start=True, stop=True)
                nc.tensor.matmul(dn[:, 128 * h:128 * (h + 1)],
                                 sb["ones_col"][:], ex[:],
                                 start=True, stop=True)
                rrow = wk.tile([1, 128], F32, tag=f"rrow{h}")
                nc.vector.reciprocal(rrow[:], dn[:, 128 * h:128 * (h + 1)])
                nc.gpsimd.partition_broadcast(rbs[h][:], rrow[:],
                                              channels=64)
            if STAGE <= 2:
                continue
            dbg_dump(f"rb{li}", rbs[0][:], 64, 128)
            dbg_dump(f"zps{li}", zps[:], 128, 128)
            z16 = wk.tile([128, 128], F16, tag="z16")
            for h in range(2):
                nc.vector.tensor_tensor(z16[64 * h:64 * (h + 1), :],
                                        zps[64 * h:64 * (h + 1), :],
                                        rbs[h][:], op=ALU.mult)
            dbg_dump(f"z16{li}", z16[:], 128, 128)
            pps = ps.tile([64, 128], F32, tag="pps")
            nc.tensor.matmul(pps[:], wlin[:], z16[:], start=True, stop=True)
            blin02 = sb["bpeff02" if li == 0 else "bmeff02"]
            ta = wk.tile([64, 128], F16, tag="ta")
            nc.scalar.activation(ta[:], pps[:], AF.Identity, bias=blin[:])
            tb = wk.tile([64, 128], F16, tag="tb")
            nc.scalar.activation(tb[:], pps[:], AF.Identity, scale=NEG,
                                 bias=blin02[:])
            if li == 0:
                nc.vector.tensor_tensor(t0_all[:, P * g:P * (g + 1)],
                                        ta[:], tb[:], op=ALU.max)
            else:
                m16 = wk.tile([64, 128], F16, tag="m16")
                nc.vector.tensor_tensor(m16[:], ta[:], tb[:], op=ALU.max)
                aps = ps.tile([8, 128], F32, tag="aps")
                nc.tensor.matmul(aps[:], sb["wnt"][:], m16[:], start=True,
                                 stop=True)
                na = wk.tile([8, 128], F16, tag="na")
                nc.scalar.activation(na[:], aps[:], AF.Identity,
                                     bias=sb["bnode"][:])
                nb = wk.tile([8, 128], F16, tag="nb")
                nc.scalar.activation(nb[:], aps[:], AF.Identity, scale=NEG,
                                     bias=sb["bnode02"][:])
                nc.vector.tensor_tensor(t1_all[:, P * g:P * (g + 1)],
                                        na[:], nb[:], op=ALU.max)

    def bn_affine(t_all, nch, gt, bt, out_t, tag):
        stat = ac.tile([nch, 2], F32, tag=f"st{tag}")
        nc.vector.reduce_sum(stat[:, 0:1], t_all[:], axis=mybir.AxisListType.X)
        junk = ac.tile([nch, NPC], F16, tag=f"junk{tag}")
        nc.scalar.activation(junk[:], t_all[:], AF.Square,
                             accum_out=stat[:, 1:2])
        ccin = dr.tile([nch, 2], F32, tag=f"ci{tag}")
        ccout = dr.tile([nch, 2], F32, tag=f"co{tag}")
        nc.sync.dma_start(ccin[:], stat[:])
        if use_cc:
            nc.gpsimd.collective_compute(
                "AllReduce", ALU.add,
                replica_groups=[list(range(N_CORES))],
                ins=[ccin.opt()], outs=[ccout.opt()])
            src = ccout
        else:
            src = ccin
        sr = ac.tile([nch, 2], F32, tag=f"sr{tag}")
        nc.sync.dma_start(sr[:], src[:])
        dbg_dump(f"sr{tag[:2]}", sr[:], nch, 2)
        dbg_dump(f"stat{tag[:2]}", stat[:], nch, 2)
        mean = ac.tile([nch, 1], F32, tag=f"mean{tag}")
        nc.scalar.activation(mean[:], sr[:, 0:1], AF.Copy, scale=1.0 / NTOT)
        msq = ac.tile([nch, 1], F32, tag=f"msq{tag}")
        nc.scalar.activation(msq[:], sr[:, 1:2], AF.Copy, scale=1.0 / NTOT)
        m2 = ac.tile([nch, 1], F32, tag=f"m2{tag}")
        nc.scalar.square(m2[:], mean[:])
        var = ac.tile([nch, 1], F32, tag=f"var{tag}")
        nc.vector.tensor_tensor(var[:], msq[:], m2[:], op=ALU.subtract)
        sd_ = ac.tile([nch, 1], F32, tag=f"sd{tag}")
        nc.scalar.activation(sd_[:], var[:], AF.Sqrt, bias=sb["epsc"][0:nch, 0:1])
        rs = ac.tile([nch, 1], F32, tag=f"rs{tag}")
        nc.vector.reciprocal(rs[:], sd_[:])
        al = ac.tile([nch, 1], F32, tag=f"al{tag}")
        nc.vector.tensor_tensor(al[:], rs[:], gt[:], op=ALU.mult)
        mt = ac.tile([nch, 1], F32, tag=f"mt{tag}")
        nc.vector.tensor_tensor(mt[:], mean[:], al[:], op=ALU.mult)
        be = ac.tile([nch, 1], F32, tag=f"be{tag}")
        nc.vector.tensor_tensor(be[:], bt[:], mt[:], op=ALU.subtract)
        nc.vector.tensor_scalar(out=out_t[:], in0=t_all[:],
                                scalar1=al[:, 0:1], scalar2=be[:, 0:1],
                                op0=ALU.mult, op1=ALU.add)

    if STAGE <= 0:
        yout = wk.tile([1, G], F32, tag="yout")
        nc.vector.tensor_copy(yout[:], sb["ident"][0:1, 0:G])
        nc.sync.dma_start(y_d.ap(), yout[:])
        return
    # layer 0
    gat_layer(0, lambda g: sb["xt16"][:, P * g:P * (g + 1)])
    if STAGE >= 90:
        return
    if STAGE <= 3:
        yout = wk.tile([1, G], F32, tag="yout")
        nc.vector.reduce_sum(yout[0:1, 0:1], t0_all[0:1, :],
                             axis=mybir.AxisListType.X)
        nc.vector.tensor_copy(yout[:, 1:G], t0_all[0:1, 1:G])
        nc.sync.dma_start(y_d.ap(), yout[:])
        return
    bn_affine(t0_all, 64, sb["bn0g"], sb["bn0b"], u0_all, f"0r{rep}")
    dbg_dump("u0", u0_all[:], 64, 1024)
    if STAGE <= 4:
        yout = wk.tile([1, G], F32, tag="yout")
        nc.vector.tensor_copy(yout[:], u0_all[0:1, 0:G])
        nc.sync.dma_start(y_d.ap(), yout[:])
        return
    # layer 1
    gat_layer(1, lambda g: u0_all[:, P * g:P * (g + 1)])
    dbg_dump("t1", t1_all[:], 8, 1024)
    bn_affine(t1_all, 8, sb["bn1g"], sb["bn1b"], a1_all, f"1r{rep}")
    dbg_dump("a1", a1_all[:], 8, 1024)

    # head
    anm = ac.tile([128, 64], F16, tag="anm")
    for g in range(G):
        atps = ps.tile([128, 8], F16, tag="hps")
        nc.tensor.transpose(atps[:], a1_all[:, P * g:P * (g + 1)],
                            sb["ident"][0:8, 0:8])
        nc.vector.tensor_copy(anm[:, 8 * g:8 * (g + 1)], atps[:])
    dbg_dump("anm", anm[:], 128, 64)
    y1ps = ps.tile([8, 256], F32, tag="fps")
    anm_r = anm[:].rearrange("p (g c) -> p c g", c=8)
    nc.tensor.matmul(y1ps[:], sb["ones_row"][:, 0:8], sb["bf1row"][:],
                     start=True, stop=False, skip_group_check=True)
    for c in range(8):
        nc.tensor.matmul(y1ps[:], anm_r[:, c:c + 1, :],
                         sb["wf1r"][:, 256 * c:256 * (c + 1)],
                         start=False, stop=(c == 7), skip_group_check=True)
    y1a = wk.tile([8, 256], F16, tag="y1a")
    nc.scalar.activation(y1a[:], y1ps[:], AF.Identity)
    y1b = wk.tile([8, 256], F16, tag="y1b")
    nc.scalar.activation(y1b[:], y1ps[:], AF.Identity, scale=NEG)
    y1 = wk.tile([8, 256], F16, tag="y1")
    nc.vector.tensor_tensor(y1[:], y1a[:], y1b[:], op=ALU.max)
    y2ps = ps.tile([32, 8], F32, tag="zps")
    for half in range(2):
        y1t = ps.tile([128, 8], F16, tag="sdt")
        nc.tensor.transpose(y1t[:], y1[:, 128 * half:128 * (half + 1)],
                            sb["ident"][0:8, 0:8])
        y1ts = wk.tile([128, 8], F16, tag=f"y1ts{half}")
        nc.vector.tensor_copy(y1ts[:], y1t[:])
        nc.tensor.matmul(y2ps[:], sb["wf2t"][:, 32 * half:32 * (half + 1)],
                         y1ts[:], start=(half == 0), stop=(half == 1))
    y2a = wk.tile([32, 8], F16, tag="y2a")
    nc.scalar.activation(y2a[:], y2ps[:], AF.Identity, bias=sb["bf2"][:])
    y2b = wk.tile([32, 8], F16, tag="y2b")
    nc.scalar.activation(y2b[:], y2ps[:], AF.Identity, scale=NEG,
                         bias=sb["bf202"][:])
    y2 = wk.tile([32, 8], F16, tag="y2")
    nc.vector.tensor_tensor(y2[:], y2a[:], y2b[:], op=ALU.max)
    y3ps = ps.tile([1, 8], F32, tag="pps")
    nc.tensor.matmul(y3ps[:], sb["wf3t"][:], y2[:], start=True, stop=True)
    yout = wk.tile([1, 8], F32, tag="yout")
    nc.scalar.activation(yout[:], y3ps[:], AF.Identity, bias=sb["bf3"][:])
    nc.sync.dma_start(y_d.ap(), yout[:])


# ---------------------------------------------------------------------------
# host side
# ---------------------------------------------------------------------------

def host_prep(inp):
    """Weights-only prep: fold biases/attention vectors, relayout, cast."""
    f = lambda k: np.asarray(inp[k], np.float32)
    w_lin0, att_src0, att_dst0 = f("w_lin0"), f("att_src0"), f("att_dst0")
    w_lin1, att_src1, att_dst1 = f("w_lin1"), f("att_src1"), f("att_dst1")
    w_post0, b_post0 = f("w_post0"), f("b_post0")
    w_mid1, b_mid1 = f("w_mid1"), f("b_mid1")
    w_node1, b_node1 = f("w_node1"), f("b_node1")
    bias0, bias1 = f("bias0"), f("bias1")
    w_f1, b_f1 = f("w_f1"), f("b_f1")
    w_f2, b_f2 = f("w_f2"), f("b_f2")
    w_f3, b_f3 = f("w_f3"), f("b_f3")

    def aug(w_lin, a_s, a_d):
        us = [a_s[h] @ w_lin[h * C:(h + 1) * C] for h in range(H)]
        ud = [a_d[h] @ w_lin[h * C:(h + 1) * C] for h in range(H)]
        cols = us + ud + [NEG * u for u in us]
        return np.concatenate([w_lin.T, np.stack(cols, 1)], 1)

    c = {}
    c["w0aug"] = aug(w_lin0, att_src0, att_dst0)
    c["w1aug"] = aug(w_lin1, att_src1, att_dst1)
    c["wpt"] = w_post0.T
    c["wmt"] = w_mid1.T
    c["wnt"] = w_node1.T
    wf1r = np.empty((128, 2048), np.float32)
    for ch in range(8):
        wf1r[:, 256 * ch:256 * (ch + 1)] = w_f1[:, ch::8].T
    c["wf1r"] = wf1r
    wf2t = np.empty((128, 64), np.float32)
    wf2t[:, 0:32] = w_f2.T[0:128]
    wf2t[:, 32:64] = w_f2.T[128:256]
    c["wf2t"] = wf2t
    c["wf3t"] = w_f3.T
    c["bf1row"] = b_f1[None, :]
    c["ones_row"] = np.ones((1, 128), np.float32)
    c["ones_col"] = np.ones((128, 1), np.float32)
    c["ident"] = np.eye(128, dtype=np.float32)
    c["bpeff"] = (b_post0 + w_post0 @ bias0)[:, None]
    c["bpeff02"] = NEG * c["bpeff"]
    c["bmeff"] = (b_mid1 + w_mid1 @ bias1)[:, None]
    c["bmeff02"] = NEG * c["bmeff"]
    c["bnode"] = b_node1[:, None]
    c["bnode02"] = NEG * c["bnode"]
    c["bf2"] = b_f2[:, None]
    c["bf202"] = NEG * c["bf2"]
    c["bf3"] = np.asarray(b_f3, np.float32).reshape(1, 1)
    c["bn0g"] = f("bn0_g")[:, None]
    c["bn0b"] = f("bn0_b")[:, None]
    c["bn1g"] = f("bn1_g")[:, None]
    c["bn1b"] = f("bn1_b")[:, None]
    c["epsc"] = np.full((64, 1), EPS, np.float32)

    out = {}
    for name, (shape, dt) in CONST_SPECS.items():
        npdt = np.float16 if dt == F16 else np.float32
        arr = np.ascontiguousarray(c[name], dtype=npdt)
        assert list(arr.shape) == shape, (name, arr.shape, shape)
        out[name] = arr
    return out


def _edge_pattern_ok(inp):
    ei = np.asarray(inp["edge_index"])
    if ei.shape != (2, B * P * P):
        return False
    jj = np.tile(np.arange(P, dtype=np.int64), P)          # src within graph
    ii = np.repeat(np.arange(P, dtype=np.int64), P)        # dst within graph
    off = (np.arange(B, dtype=np.int64) * P)[:, None]
    src = (off + jj[None]).reshape(-1)
    dst = (off + ii[None]).reshape(-1)
    bok = np.array_equal(np.asarray(inp["batch"]).ravel(),
                         np.arange(B * P) // P)
    return (np.array_equal(ei[0].astype(np.int64), src)
            and np.array_equal(ei[1].astype(np.int64), dst) and bok)


def _numpy_reference(inp):
    """General fallback (arbitrary edges), pure numpy."""
    f = lambda k: np.asarray(inp[k], np.float32)
    x = f("x")
    n = x.shape[0]
    src = np.asarray(inp["edge_index"])[0].astype(np.int64)
    dst = np.asarray(inp["edge_index"])[1].astype(np.int64)

    def lrelu(v):
        return np.where(v > 0, v, NEG * v)

    def gat(h_in, w_lin, a_s, a_d, bias):
        h = (h_in @ w_lin.T).reshape(n, H, C)
        s = np.einsum("nhc,hc->nh", h, a_s)
        d = np.einsum("nhc,hc->nh", h, a_d)
        e = lrelu(s[src] + d[dst])
        m = np.full((n, H), -np.inf, np.float32)
        np.maximum.at(m, dst, e)
        ex = np.exp(e - m[dst])
        den = np.zeros((n, H), np.float32)
        np.add.at(den, dst, ex)
        attn = ex / den[dst]
        msg = h[src] * attn[:, :, None]
        out = np.zeros((n, H, C), np.float32)
        np.add.at(out, dst, msg)
        return out.reshape(n, H * C) + bias

    def bn(z, g_, b_):
        mu = z.mean(0)
        v = z.var(0)
        return (z - mu) / np.sqrt(v + EPS) * g_ + b_

    z = gat(x, f("w_lin0"), f("att_src0"), f("att_dst0"), f("bias0"))
    z = bn(lrelu(z @ f("w_post0").T + f("b_post0")), f("bn0_g"), f("bn0_b"))
    z = gat(z, f("w_lin1"), f("att_src1"), f("att_dst1"), f("bias1"))
    z = lrelu(z @ f("w_mid1").T + f("b_mid1"))
    z = bn(lrelu(z @ f("w_node1").T + f("b_node1")), f("bn1_g"), f("bn1_b"))
    z = z.reshape(n // P, -1)
    z = lrelu(z @ f("w_f1").T + f("b_f1"))
    z = lrelu(z @ f("w_f2").T + f("b_f2"))
    return z @ f("w_f3").T + f("b_f3")


_CACHE = {}


def get_program(use_cc=True, reps=1):
    key = (use_cc, reps)
    if key not in _CACHE:
        _CACHE[key] = build_program(use_cc=use_cc, reps=reps)
    return _CACHE[key]


def make_in_maps(inputs):
    consts = host_prep(inputs)
    x = np.asarray(inputs["x"], np.float32)
    in_maps = []
    for k in range(N_CORES):
        shard = x[NPC * k:NPC * (k + 1)]           # [1024, 128]
        xt16 = np.ascontiguousarray(shard.T, dtype=np.float16)
        m = {"xt16": xt16}
        m.update(consts)
        in_maps.append(m)
    return in_maps


def kernel(**inputs):
    if not _edge_pattern_ok(inputs):
        return _numpy_reference(inputs).astype(np.float32)
    nc = get_program(use_cc=True, reps=1)
    in_maps = make_in_maps(inputs)
    res = bass_utils.run_bass_kernel_spmd(nc, in_maps,
                                          core_ids=list(range(N_CORES)))
    y = np.empty((B, 1), np.float32)
    for k in range(N_CORES):
        y[G * k:G * (k + 1), 0] = res.results[k]["y"][0]
    return y
